# revision 9
# baseline (speedup 1.0000x reference)
"""Trainium2 Bass kernel: conv2x2 + LSTM actor-critic feature trunk.

Full inputs in, full outputs out. Data-parallel over batch: 8 NeuronCores,
32 sequences per core. Per core:
  Phase A: dense-matmul conv (bias folded via ones-row) -> relu -> einsum
           producing gate pre-activations gx[4H, T*32] kept in SBUF.
  Phase B: 512 serial LSTM steps, hidden state stored [H=64 part, B=32 free]
           so the recurrent matmul needs no per-step transpose; gx is
           preloaded into PSUM and the matmul accumulates onto it.
"""

import sys
import numpy as np

for _p in ("/opt/trn_rl_repo",):
    if _p not in sys.path:
        sys.path.insert(0, _p)

HID = 64
NUM_ACT = 4
B, T = 256, 512
NCORES = 8
BL = B // NCORES          # 32 sequences per core
S = BL * T                # 16384 samples per core, t-major: s = t*BL + b
NT = 512                  # samples per phase-A tile
NTILES = S // NT
FIN = 7 * 7 * 4           # 196 input features per sample
FCONV = 8 * 6 * 6         # 288 conv output features
FEAT = FCONV + NUM_ACT + 2  # 294
KA = 128                  # phase-A K chunk split of FIN+1: 128 + 69
KB = FIN + 1 - KA
CHUNK = 64                # LSTM steps per output DMA chunk

USE_PSUM_PRELOAD = True

_cache = {}


def _build():
    from concourse import bacc, tile, mybir

    f32 = mybir.dt.float32
    AF = mybir.ActivationFunctionType

    nc = bacc.Bacc(None, target_bir_lowering=False, debug=False)

    xT_d = nc.declare_dram_parameter("xT", [FIN + 1, S], f32, isOutput=False)
    ex_d = nc.declare_dram_parameter("extras", [7, S], f32, isOutput=False)
    wca_d = nc.declare_dram_parameter("wc_a", [KA, FCONV], f32, isOutput=False)
    wcb_d = nc.declare_dram_parameter("wc_b", [KB, FCONV], f32, isOutput=False)
    wih0_d = nc.declare_dram_parameter("wih_0", [128, 256], f32, isOutput=False)
    wih1_d = nc.declare_dram_parameter("wih_1", [128, 256], f32, isOutput=False)
    wih2_d = nc.declare_dram_parameter("wih_2", [39, 256], f32, isOutput=False)
    whht_d = nc.declare_dram_parameter("whh_t", [HID, 128], f32, isOutput=False)
    whhb_d = nc.declare_dram_parameter("whh_b", [HID, 128], f32, isOutput=False)
    h0_d = nc.declare_dram_parameter("h0", [HID, BL], f32, isOutput=False)
    c0_d = nc.declare_dram_parameter("c0", [HID, BL], f32, isOutput=False)
    fid_d = nc.declare_dram_parameter("fold_id", [128, HID], f32, isOutput=False)
    feat_d = nc.declare_dram_parameter("feat_out", [HID, S], f32, isOutput=True)
    h_out_d = nc.declare_dram_parameter("h_out", [HID, BL], f32, isOutput=True)
    c_out_d = nc.declare_dram_parameter("c_out", [HID, BL], f32, isOutput=True)

    TPA = NT // BL  # LSTM steps covered by one phase-A tile (16)

    with tile.TileContext(nc) as tc:
        with (
            tc.tile_pool(name="const", bufs=1) as cpool,
            tc.tile_pool(name="state", bufs=1) as spool,
        ):
            # --- weights / constants, loaded once ---
            wca = cpool.tile([KA, FCONV], f32)
            wcb = cpool.tile([KB, FCONV], f32)
            wih0 = cpool.tile([128, 256], f32)
            wih1 = cpool.tile([128, 256], f32)
            wih2 = cpool.tile([39, 256], f32)
            whht = cpool.tile([HID, 128], f32)
            whhb = cpool.tile([HID, 128], f32)
            h0 = cpool.tile([HID, BL], f32)
            fid = cpool.tile([128, HID], f32)
            nc.sync.dma_start(out=fid[:], in_=fid_d[:])
            nc.sync.dma_start(out=wca[:], in_=wca_d[:])
            nc.sync.dma_start(out=wcb[:], in_=wcb_d[:])
            nc.sync.dma_start(out=wih0[:], in_=wih0_d[:])
            nc.sync.dma_start(out=wih1[:], in_=wih1_d[:])
            nc.sync.dma_start(out=wih2[:], in_=wih2_d[:])
            nc.sync.dma_start(out=whht[:], in_=whht_d[:])
            nc.sync.dma_start(out=whhb[:], in_=whhb_d[:])
            nc.sync.dma_start(out=h0[:], in_=h0_d[:])

            # c (rows 0:64, persistent) stacked over g~ (rows 64:128, per step)
            gc = spool.tile([128, BL], f32)
            nc.sync.dma_start(out=gc[0:HID, :], in_=c0_d[:])

            # Phase A (conv + einsum -> gx in PSUM) interleaved into phase B's
            # stall windows. gx for each 16-step window lives in a PSUM tile;
            # the step matmuls accumulate straight onto it (no preload copies).
            from collections import deque

            with (
                tc.tile_pool(name="xa", bufs=2) as xapool,
                tc.tile_pool(name="xb", bufs=2) as xbpool,
                tc.tile_pool(name="f01", bufs=2) as fpool,
                tc.tile_pool(name="f2", bufs=2) as f2pool,
                tc.tile_pool(name="pc", bufs=1, space="PSUM") as pcpool,
                tc.tile_pool(name="pgt", bufs=3, space="PSUM") as pgtpool,
                tc.tile_pool(name="pgb", bufs=3, space="PSUM") as pgbpool,
                tc.tile_pool(name="hs", bufs=2) as hspool,
                tc.tile_pool(name="sb", bufs=3) as sbpool,
                tc.tile_pool(name="pcn", bufs=1, space="PSUM") as pcnpool,
            ):
                gx_tiles = {}

                def a_tile_ops(ta):
                    n0 = ta * NT
                    xa = xapool.tile([KA, NT], f32, tag="xa")
                    xb = xbpool.tile([KB, NT], f32, tag="xb")
                    f0 = fpool.tile([128, NT], f32, tag="f0")
                    f1 = fpool.tile([128, NT], f32, tag="f1")
                    f2 = f2pool.tile([39, NT], f32, tag="f2")
                    pgt = pgtpool.tile([128, NT], f32, tag="pgt")
                    pgb = pgbpool.tile([128, NT], f32, tag="pgb")
                    gx_tiles[ta] = (pgt, pgb)
                    ops = []
                    ops.append(lambda: nc.sync.dma_start(
                        out=xa[:], in_=xT_d[0:KA, n0:n0 + NT]))
                    ops.append(lambda: nc.sync.dma_start(
                        out=xb[:], in_=xT_d[KA:KA + KB, n0:n0 + NT]))
                    ops.append(lambda: nc.sync.dma_start(
                        out=f2[32:39, :], in_=ex_d[:, n0:n0 + NT]))
                    for (m0, mc, dst) in ((0, 128, f0), (128, 128, f1),
                                          (256, 32, None)):
                        pc = pcpool.tile([mc, NT], f32, tag="pc")
                        d = dst[:, :] if dst is not None else f2[0:32, :]
                        ops.append(lambda pc=pc, m0=m0, mc=mc: nc.tensor.matmul(
                            pc[:], wca[:, m0:m0 + mc], xa[:],
                            start=True, stop=False))
                        ops.append(lambda pc=pc, m0=m0, mc=mc: nc.tensor.matmul(
                            pc[:], wcb[:, m0:m0 + mc], xb[:],
                            start=False, stop=True))
                        ops.append(lambda pc=pc, d=d: nc.scalar.activation(
                            d, pc[:], AF.Relu))
                    for gh, pg in ((0, pgt), (1, pgb)):
                        c0_ = gh * 128
                        ops.append(lambda pg=pg, c0_=c0_: nc.tensor.matmul(
                            pg[:], wih0[:, c0_:c0_ + 128], f0[:],
                            start=True, stop=False))
                        ops.append(lambda pg=pg, c0_=c0_: nc.tensor.matmul(
                            pg[:], wih1[:, c0_:c0_ + 128], f1[:],
                            start=False, stop=False))
                        ops.append(lambda pg=pg, c0_=c0_: nc.tensor.matmul(
                            pg[:], wih2[:, c0_:c0_ + 128], f2[:],
                            start=False, stop=True))
                    return ops

                # prologue: first two phase-A tiles fully emitted up front
                for op in a_tile_ops(0) + a_tile_ops(1):
                    op()

                aq = deque()
                h_prev = h0
                hs = None
                for t in range(T):
                    ta_now = t // TPA
                    if t % TPA == 0 and ta_now + 2 < NTILES:
                        aq.extend(a_tile_ops(ta_now + 2))

                    sl = t % CHUNK
                    if sl == 0:
                        hs = hspool.tile([HID, CHUNK * BL], f32, tag="hs")
                    pgt, pgb = gx_tiles[ta_now]
                    s0 = (t % TPA) * BL
                    g_t = pgt[:, s0:s0 + BL]
                    g_b = pgb[:, s0:s0 + BL]

                    # gates: top = [f | i], bot = [o | g]  (host-permuted)
                    nc.tensor.matmul(g_b, whhb[:], h_prev,
                                     start=False, stop=True,
                                     skip_group_check=True)
                    nc.tensor.matmul(g_t, whht[:], h_prev,
                                     start=False, stop=True,
                                     skip_group_check=True)

                    # g~ = tanh(g) into gc rows 64:128 (c sits in rows 0:64)
                    nc.scalar.activation(gc[HID:, :], pgb[HID:, s0:s0 + BL],
                                         AF.Tanh)
                    sig_fi = sbpool.tile([128, BL], f32, tag="sfi")
                    nc.scalar.activation(sig_fi[:], g_t, AF.Sigmoid)
                    sig_o = sbpool.tile([HID, BL], f32, tag="so")
                    nc.scalar.activation(sig_o[:], pgb[0:HID, s0:s0 + BL],
                                         AF.Sigmoid)

                    # prod = [f*c | i*g~]; fold halves with [I;I] matmul -> c_new
                    prod = sbpool.tile([128, BL], f32, tag="prod")
                    nc.vector.tensor_mul(prod[:], sig_fi[:], gc[:])
                    pcn = pcnpool.tile([HID, BL], f32, tag="pcn")
                    nc.tensor.matmul(pcn[:], fid[:], prod[:], start=True,
                                     stop=True)

                    tc_ = sbpool.tile([HID, BL], f32, tag="tc")
                    nc.scalar.activation(tc_[:], pcn[:], AF.Tanh)
                    hsl = hs[:, sl * BL:(sl + 1) * BL]
                    nc.vector.tensor_mul(hsl, sig_o[:], tc_[:])
                    nc.vector.tensor_copy(gc[0:HID, :], pcn[:])
                    h_prev = hsl

                    if sl == CHUNK - 1:
                        c0o = (t - sl) * BL
                        nc.sync.dma_start(out=feat_d[:, c0o:c0o + CHUNK * BL],
                                          in_=hs[:])

                    npop = 2 if len(aq) > 24 else 1
                    for _ in range(npop):
                        if aq:
                            aq.popleft()()
                while aq:
                    aq.popleft()()

                nc.sync.dma_start(out=h_out_d[:], in_=h_prev)
                nc.sync.dma_start(out=c_out_d[:], in_=gc[0:HID, :])

    nc.compile()
    return nc


def _pack_weights(conv_w, conv_b, w_ih, w_hh, b_ih, b_hh):
    f32 = np.float32
    wc = np.zeros((FIN + 1, FCONV), dtype=f32)
    cw = np.asarray(conv_w, dtype=f32)
    for co in range(8):
        for ci in range(4):
            for di in range(2):
                for dj in range(2):
                    v = cw[co, ci, di, dj]
                    for io in range(6):
                        for jo in range(6):
                            f_in = (io + di) * 28 + (jo + dj) * 4 + ci
                            f_out = co * 36 + io * 6 + jo
                            wc[f_in, f_out] += v
    wc[FIN, :] = np.repeat(np.asarray(conv_b, dtype=f32), 36)

    # gate-row permutation: torch order [i,f,g,o] -> device order [f,i | o,g]
    perm = np.r_[64:128, 0:64, 192:256, 128:192]
    wih_p = np.asarray(w_ih, dtype=f32)[perm]                         # [256, 294]
    whh_p = np.asarray(w_hh, dtype=f32)[perm]                         # [256, 64]
    bias = (np.asarray(b_ih, dtype=f32) + np.asarray(b_hh, dtype=f32))[perm]
    wihT = np.ascontiguousarray(wih_p.T)                              # [294, 256]
    wih2 = np.concatenate([wihT[256:294], bias[None, :]], axis=0)     # [39, 256]
    whhT = np.ascontiguousarray(whh_p.T)                              # [64, 256]
    fold_id = np.concatenate([np.eye(HID, dtype=f32),
                              np.eye(HID, dtype=f32)], axis=0)        # [128, 64]
    return {
        "wc_a": np.ascontiguousarray(wc[0:KA]),
        "wc_b": np.ascontiguousarray(wc[KA:]),
        "wih_0": np.ascontiguousarray(wihT[0:128]),
        "wih_1": np.ascontiguousarray(wihT[128:256]),
        "wih_2": np.ascontiguousarray(wih2),
        "whh_t": np.ascontiguousarray(whhT[:, 0:128]),
        "whh_b": np.ascontiguousarray(whhT[:, 128:256]),
        "fold_id": fold_id,
    }


def kernel(x, hidden, prev_action, prev_reward, prev_done,
           conv_w, conv_b, w_ih, w_hh, b_ih, b_hh):
    from concourse import bass_utils

    x = np.asarray(x, dtype=np.float32)
    hidden = np.asarray(hidden, dtype=np.float32)
    prev_action = np.asarray(prev_action)
    prev_reward = np.asarray(prev_reward, dtype=np.float32)
    prev_done = np.asarray(prev_done, dtype=np.float32)

    wpack = _pack_weights(conv_w, conv_b, w_ih, w_hh, b_ih, b_hh)

    in_maps = []
    for m in range(NCORES):
        b0 = m * BL
        xm = x[b0:b0 + BL].reshape(BL, T, FIN)
        xTm = np.empty((FIN + 1, S), dtype=np.float32)
        xTm[0:FIN] = xm.transpose(2, 1, 0).reshape(FIN, S)
        xTm[FIN] = 1.0

        pa = prev_action[b0:b0 + BL].T.reshape(-1)          # [S] t-major
        ex = np.empty((7, S), dtype=np.float32)
        ex[0:4] = (np.arange(NUM_ACT)[:, None] == pa[None, :])
        ex[4] = prev_reward[b0:b0 + BL].T.reshape(-1)
        ex[5] = prev_done[b0:b0 + BL].T.reshape(-1)
        ex[6] = 1.0

        hm = hidden[b0:b0 + BL]
        im = {
            "xT": xTm,
            "extras": ex,
            "h0": np.ascontiguousarray(hm[:, 0:HID].T),
            "c0": np.ascontiguousarray(hm[:, HID:].T),
        }
        im.update(wpack)
        in_maps.append(im)

    if "nc" not in _cache:
        _cache["nc"] = _build()
    nc = _cache["nc"]

    res = bass_utils.run_bass_kernel_spmd(nc, in_maps, core_ids=list(range(NCORES)))

    features = np.empty((B, T, HID), dtype=np.float32)
    hidden_out = np.empty((B, 2 * HID), dtype=np.float32)
    for m in range(NCORES):
        b0 = m * BL
        fo = res.results[m]["feat_out"]                     # [64, S]
        features[b0:b0 + BL] = fo.reshape(HID, T, BL).transpose(2, 1, 0)
        hidden_out[b0:b0 + BL, 0:HID] = res.results[m]["h_out"].T
        hidden_out[b0:b0 + BL, HID:] = res.results[m]["c_out"].T
    return features, hidden_out


# revision 13
# speedup vs baseline: 1.2546x; 1.2546x over previous
"""Trainium2 Bass kernel: conv2x2 + LSTM actor-critic feature trunk.

Full inputs in, full outputs out. Data-parallel over batch: 8 NeuronCores,
32 sequences per core. Per core:
  Phase A: dense-matmul conv (bias folded via ones-row) -> relu -> einsum
           producing gate pre-activations gx[4H, T*32] kept in SBUF.
  Phase B: 512 serial LSTM steps, hidden state stored [H=64 part, B=32 free]
           so the recurrent matmul needs no per-step transpose; gx is
           preloaded into PSUM and the matmul accumulates onto it.
"""

import sys
import numpy as np

for _p in ("/opt/trn_rl_repo",):
    if _p not in sys.path:
        sys.path.insert(0, _p)

HID = 64
NUM_ACT = 4
B, T = 256, 512
NCORES = 8
BL = B // NCORES          # 32 sequences per core
S = BL * T                # 16384 samples per core, t-major: s = t*BL + b
NT = 512                  # samples per phase-A tile
NTILES = S // NT
FIN = 7 * 7 * 4           # 196 input features per sample
FCONV = 8 * 6 * 6         # 288 conv output features
FEAT = FCONV + NUM_ACT + 2  # 294
KA = 128                  # phase-A K chunk split of FIN+1: 128 + 69
KB = FIN + 1 - KA
CHUNK = 64                # LSTM steps per output DMA chunk

USE_PSUM_PRELOAD = True

_cache = {}


def _build():
    from concourse import bacc, tile, mybir

    f32 = mybir.dt.float32
    AF = mybir.ActivationFunctionType

    nc = bacc.Bacc(None, target_bir_lowering=False, debug=False)

    xT_d = nc.declare_dram_parameter("xT", [FIN + 1, S], f32, isOutput=False)
    ex_d = nc.declare_dram_parameter("extras", [7, S], f32, isOutput=False)
    wca_d = nc.declare_dram_parameter("wc_a", [KA, FCONV], f32, isOutput=False)
    wcb_d = nc.declare_dram_parameter("wc_b", [KB, FCONV], f32, isOutput=False)
    wih0_d = nc.declare_dram_parameter("wih_0", [128, 256], f32, isOutput=False)
    wih1_d = nc.declare_dram_parameter("wih_1", [128, 256], f32, isOutput=False)
    wih2_d = nc.declare_dram_parameter("wih_2", [39, 256], f32, isOutput=False)
    whht_d = nc.declare_dram_parameter("whh_t", [HID, 128], f32, isOutput=False)
    whhb_d = nc.declare_dram_parameter("whh_b", [HID, 128], f32, isOutput=False)
    h0_d = nc.declare_dram_parameter("h0", [HID, BL], f32, isOutput=False)
    c0_d = nc.declare_dram_parameter("c0", [HID, BL], f32, isOutput=False)
    fid_d = nc.declare_dram_parameter("fold_id", [128, HID], f32, isOutput=False)
    feat_d = nc.declare_dram_parameter("feat_out", [HID, S], f32, isOutput=True)
    h_out_d = nc.declare_dram_parameter("h_out", [HID, BL], f32, isOutput=True)
    c_out_d = nc.declare_dram_parameter("c_out", [HID, BL], f32, isOutput=True)

    with tile.TileContext(nc) as tc:
        with (
            tc.tile_pool(name="const", bufs=1) as cpool,
            tc.tile_pool(name="gx", bufs=1) as gxpool,
            tc.tile_pool(name="state", bufs=1) as spool,
        ):
            # --- weights / constants, loaded once ---
            wca = cpool.tile([KA, FCONV], f32)
            wcb = cpool.tile([KB, FCONV], f32)
            wih0 = cpool.tile([128, 256], f32)
            wih1 = cpool.tile([128, 256], f32)
            wih2 = cpool.tile([39, 256], f32)
            whht = cpool.tile([HID, 128], f32)
            whhb = cpool.tile([HID, 128], f32)
            h0 = cpool.tile([HID, BL], f32)
            fid = cpool.tile([128, HID], f32)
            nc.sync.dma_start(out=fid[:], in_=fid_d[:])
            nc.sync.dma_start(out=wca[:], in_=wca_d[:])
            nc.sync.dma_start(out=wcb[:], in_=wcb_d[:])
            nc.sync.dma_start(out=wih0[:], in_=wih0_d[:])
            nc.sync.dma_start(out=wih1[:], in_=wih1_d[:])
            nc.sync.dma_start(out=wih2[:], in_=wih2_d[:])
            nc.sync.dma_start(out=whht[:], in_=whht_d[:])
            nc.sync.dma_start(out=whhb[:], in_=whhb_d[:])
            nc.sync.dma_start(out=h0[:], in_=h0_d[:])

            # persistent gate pre-activations for the whole sequence
            gx_top = gxpool.tile([128, S], f32)   # gates i,f
            gx_bot = gxpool.tile([128, S], f32)   # gates g,o

            # c (rows 0:64, persistent) stacked over g~ (rows 64:128, per step)
            gc = spool.tile([128, BL], f32)
            nc.sync.dma_start(out=gc[0:HID, :], in_=c0_d[:])

            # ---------------- Phase A: conv + einsum -> gx ----------------
            with (
                tc.tile_pool(name="xa", bufs=3) as xapool,
                tc.tile_pool(name="xb", bufs=3) as xbpool,
                tc.tile_pool(name="f01", bufs=2) as fpool,
                tc.tile_pool(name="f2", bufs=2) as f2pool,
                tc.tile_pool(name="pc", bufs=2, space="PSUM") as pcpool,
                tc.tile_pool(name="pg", bufs=2, space="PSUM") as pgpool,
            ):
                for it in range(NTILES):
                    n0 = it * NT
                    xa = xapool.tile([KA, NT], f32)
                    xb = xbpool.tile([KB, NT], f32)
                    nc.sync.dma_start(out=xa[:], in_=xT_d[0:KA, n0:n0 + NT])
                    nc.sync.dma_start(out=xb[:], in_=xT_d[KA:KA + KB, n0:n0 + NT])

                    f0 = fpool.tile([128, NT], f32, tag="f0")
                    f1 = fpool.tile([128, NT], f32, tag="f1")
                    f2 = f2pool.tile([39, NT], f32)
                    nc.sync.dma_start(out=f2[32:39, :], in_=ex_d[:, n0:n0 + NT])

                    # conv output M chunks: 0:128 -> f0, 128:256 -> f1, 256:288 -> f2[0:32]
                    for (m0, mc, dst) in ((0, 128, f0[:, :]), (128, 128, f1[:, :]),
                                          (256, 32, f2[0:32, :])):
                        pc = pcpool.tile([mc, NT], f32, tag="pc")
                        nc.tensor.matmul(pc[:], wca[:, m0:m0 + mc], xa[:],
                                         start=True, stop=False)
                        nc.tensor.matmul(pc[:], wcb[:, m0:m0 + mc], xb[:],
                                         start=False, stop=True)
                        nc.scalar.activation(dst, pc[:], AF.Relu)

                    for gh, gdst in ((0, gx_top), (1, gx_bot)):
                        pg = pgpool.tile([128, NT], f32, tag="pg")
                        c0_ = gh * 128
                        nc.tensor.matmul(pg[:], wih0[:, c0_:c0_ + 128], f0[:],
                                         start=True, stop=False)
                        nc.tensor.matmul(pg[:], wih1[:, c0_:c0_ + 128], f1[:],
                                         start=False, stop=False)
                        nc.tensor.matmul(pg[:], wih2[:, c0_:c0_ + 128], f2[:],
                                         start=False, stop=True)
                        nc.vector.tensor_copy(gdst[:, n0:n0 + NT], pg[:])

            # ---------------- Phase B: serial LSTM ----------------
            with (
                tc.tile_pool(name="hs", bufs=2) as hspool,
                tc.tile_pool(name="sb", bufs=3) as sbpool,
                tc.tile_pool(name="pt", bufs=3, space="PSUM") as ptpool,
                tc.tile_pool(name="pb", bufs=3, space="PSUM") as pbpool,
                tc.tile_pool(name="pc", bufs=2, space="PSUM") as pcnpool,
            ):
                h_prev = h0
                hs = None
                for t in range(T):
                    sl = t % CHUNK
                    if sl == 0:
                        hs = hspool.tile([HID, CHUNK * BL], f32, tag="hs")
                    cl = t * BL

                    # gates: top = [f | i], bot = [o | g]  (host-permuted)
                    pt = ptpool.tile([128, BL], f32, tag="pt")
                    pb = pbpool.tile([128, BL], f32, tag="pb")
                    if USE_PSUM_PRELOAD:
                        nc.vector.tensor_copy(pb[:], gx_bot[:, cl:cl + BL])
                        nc.vector.tensor_copy(pt[:], gx_top[:, cl:cl + BL])
                        # bot first: tanh(g) can start while the top MM runs
                        nc.tensor.matmul(pb[:], whhb[:], h_prev,
                                         start=False, stop=True,
                                         skip_group_check=True)
                        nc.tensor.matmul(pt[:], whht[:], h_prev,
                                         start=False, stop=True,
                                         skip_group_check=True)
                        g_t, g_b = pt, pb
                    else:
                        nc.tensor.matmul(pt[:], whht[:], h_prev,
                                         start=True, stop=True)
                        nc.tensor.matmul(pb[:], whhb[:], h_prev,
                                         start=True, stop=True)
                        at = sbpool.tile([128, BL], f32, tag="at")
                        ab = sbpool.tile([128, BL], f32, tag="ab")
                        nc.vector.tensor_add(at[:], pt[:], gx_top[:, cl:cl + BL])
                        nc.vector.tensor_add(ab[:], pb[:], gx_bot[:, cl:cl + BL])
                        g_t, g_b = at, ab

                    # g~ = tanh(g) into gc rows 64:128 (c sits in rows 0:64)
                    nc.scalar.activation(gc[HID:, :], g_b[HID:, :], AF.Tanh)
                    sig_fi = sbpool.tile([128, BL], f32, tag="sfi")
                    nc.scalar.activation(sig_fi[:], g_t[:], AF.Sigmoid)
                    sig_o = sbpool.tile([HID, BL], f32, tag="so")
                    nc.scalar.activation(sig_o[:], g_b[0:HID, :], AF.Sigmoid)

                    # prod = [f*c | i*g~]; fold halves with [I;I] matmul -> c_new
                    prod = sbpool.tile([128, BL], f32, tag="prod")
                    nc.vector.tensor_mul(prod[:], sig_fi[:], gc[:])
                    pcn = pcnpool.tile([HID, BL], f32, tag="pcn")
                    nc.tensor.matmul(pcn[:], fid[:], prod[:], start=True, stop=True)

                    tc_ = sbpool.tile([HID, BL], f32, tag="tc")
                    nc.scalar.activation(tc_[:], pcn[:], AF.Tanh)
                    hsl = hs[:, sl * BL:(sl + 1) * BL]
                    nc.vector.tensor_mul(hsl, sig_o[:], tc_[:])
                    nc.vector.tensor_copy(gc[0:HID, :], pcn[:])
                    h_prev = hsl

                    if sl == CHUNK - 1:
                        c0o = (t - sl) * BL
                        nc.sync.dma_start(out=feat_d[:, c0o:c0o + CHUNK * BL],
                                          in_=hs[:])

                nc.sync.dma_start(out=h_out_d[:], in_=h_prev)
                nc.sync.dma_start(out=c_out_d[:], in_=gc[0:HID, :])

    nc.compile()
    return nc


def _pack_weights(conv_w, conv_b, w_ih, w_hh, b_ih, b_hh):
    f32 = np.float32
    wc = np.zeros((FIN + 1, FCONV), dtype=f32)
    cw = np.asarray(conv_w, dtype=f32)
    for co in range(8):
        for ci in range(4):
            for di in range(2):
                for dj in range(2):
                    v = cw[co, ci, di, dj]
                    for io in range(6):
                        for jo in range(6):
                            f_in = (io + di) * 28 + (jo + dj) * 4 + ci
                            f_out = co * 36 + io * 6 + jo
                            wc[f_in, f_out] += v
    wc[FIN, :] = np.repeat(np.asarray(conv_b, dtype=f32), 36)

    # gate-row permutation: torch order [i,f,g,o] -> device order [f,i | o,g]
    perm = np.r_[64:128, 0:64, 192:256, 128:192]
    wih_p = np.asarray(w_ih, dtype=f32)[perm]                         # [256, 294]
    whh_p = np.asarray(w_hh, dtype=f32)[perm]                         # [256, 64]
    bias = (np.asarray(b_ih, dtype=f32) + np.asarray(b_hh, dtype=f32))[perm]
    wihT = np.ascontiguousarray(wih_p.T)                              # [294, 256]
    wih2 = np.concatenate([wihT[256:294], bias[None, :]], axis=0)     # [39, 256]
    whhT = np.ascontiguousarray(whh_p.T)                              # [64, 256]
    fold_id = np.concatenate([np.eye(HID, dtype=f32),
                              np.eye(HID, dtype=f32)], axis=0)        # [128, 64]
    return {
        "wc_a": np.ascontiguousarray(wc[0:KA]),
        "wc_b": np.ascontiguousarray(wc[KA:]),
        "wih_0": np.ascontiguousarray(wihT[0:128]),
        "wih_1": np.ascontiguousarray(wihT[128:256]),
        "wih_2": np.ascontiguousarray(wih2),
        "whh_t": np.ascontiguousarray(whhT[:, 0:128]),
        "whh_b": np.ascontiguousarray(whhT[:, 128:256]),
        "fold_id": fold_id,
    }


def kernel(x, hidden, prev_action, prev_reward, prev_done,
           conv_w, conv_b, w_ih, w_hh, b_ih, b_hh):
    from concourse import bass_utils

    x = np.asarray(x, dtype=np.float32)
    hidden = np.asarray(hidden, dtype=np.float32)
    prev_action = np.asarray(prev_action)
    prev_reward = np.asarray(prev_reward, dtype=np.float32)
    prev_done = np.asarray(prev_done, dtype=np.float32)

    wpack = _pack_weights(conv_w, conv_b, w_ih, w_hh, b_ih, b_hh)

    in_maps = []
    for m in range(NCORES):
        b0 = m * BL
        xm = x[b0:b0 + BL].reshape(BL, T, FIN)
        xTm = np.empty((FIN + 1, S), dtype=np.float32)
        xTm[0:FIN] = xm.transpose(2, 1, 0).reshape(FIN, S)
        xTm[FIN] = 1.0

        pa = prev_action[b0:b0 + BL].T.reshape(-1)          # [S] t-major
        ex = np.empty((7, S), dtype=np.float32)
        ex[0:4] = (np.arange(NUM_ACT)[:, None] == pa[None, :])
        ex[4] = prev_reward[b0:b0 + BL].T.reshape(-1)
        ex[5] = prev_done[b0:b0 + BL].T.reshape(-1)
        ex[6] = 1.0

        hm = hidden[b0:b0 + BL]
        im = {
            "xT": xTm,
            "extras": ex,
            "h0": np.ascontiguousarray(hm[:, 0:HID].T),
            "c0": np.ascontiguousarray(hm[:, HID:].T),
        }
        im.update(wpack)
        in_maps.append(im)

    if "nc" not in _cache:
        _cache["nc"] = _build()
    nc = _cache["nc"]

    res = bass_utils.run_bass_kernel_spmd(nc, in_maps, core_ids=list(range(NCORES)))

    features = np.empty((B, T, HID), dtype=np.float32)
    hidden_out = np.empty((B, 2 * HID), dtype=np.float32)
    for m in range(NCORES):
        b0 = m * BL
        fo = res.results[m]["feat_out"]                     # [64, S]
        features[b0:b0 + BL] = fo.reshape(HID, T, BL).transpose(2, 1, 0)
        hidden_out[b0:b0 + BL, 0:HID] = res.results[m]["h_out"].T
        hidden_out[b0:b0 + BL, HID:] = res.results[m]["c_out"].T
    return features, hidden_out


# revision 15
# speedup vs baseline: 1.4711x; 1.1726x over previous
"""Trainium2 Bass kernel: conv2x2 + LSTM actor-critic feature trunk.

Full inputs in, full outputs out. Data-parallel over batch: 8 NeuronCores,
32 sequences per core. Per core:
  Phase A: dense-matmul conv (bias folded via ones-row) -> relu -> einsum
           producing gate pre-activations gx[4H, T*32] kept in SBUF.
  Phase B: 512 serial LSTM steps, hidden state stored [H=64 part, B=32 free]
           so the recurrent matmul needs no per-step transpose; gx is
           preloaded into PSUM and the matmul accumulates onto it.
"""

import sys
import numpy as np
import ml_dtypes

BF16 = ml_dtypes.bfloat16

for _p in ("/opt/trn_rl_repo",):
    if _p not in sys.path:
        sys.path.insert(0, _p)

HID = 64
NUM_ACT = 4
B, T = 256, 512
NCORES = 8
BL = B // NCORES          # 32 sequences per core
S = BL * T                # 16384 samples per core, t-major: s = t*BL + b
NT = 512                  # samples per phase-A tile
NTILES = S // NT
FIN = 7 * 7 * 4           # 196 input features per sample
FCONV = 8 * 6 * 6         # 288 conv output features
FEAT = FCONV + NUM_ACT + 2  # 294
KA = 128                  # phase-A K chunk split of FIN+1: 128 + 69
KB = FIN + 1 - KA
CHUNK = 64                # LSTM steps per output DMA chunk

USE_PSUM_PRELOAD = True

_cache = {}


def _build():
    from concourse import bacc, tile, mybir

    f32 = mybir.dt.float32
    bf16 = mybir.dt.bfloat16
    AF = mybir.ActivationFunctionType

    nc = bacc.Bacc(None, target_bir_lowering=False, debug=False)

    xT_d = nc.declare_dram_parameter("xT", [FIN + 1, S], bf16, isOutput=False)
    ex_d = nc.declare_dram_parameter("extras", [7, S], bf16, isOutput=False)
    wca_d = nc.declare_dram_parameter("wc_a", [KA, FCONV], bf16, isOutput=False)
    wcb_d = nc.declare_dram_parameter("wc_b", [KB, FCONV], bf16, isOutput=False)
    wih0_d = nc.declare_dram_parameter("wih_0", [128, 256], bf16, isOutput=False)
    wih1_d = nc.declare_dram_parameter("wih_1", [128, 256], bf16, isOutput=False)
    wih2_d = nc.declare_dram_parameter("wih_2", [39, 256], bf16, isOutput=False)
    whht_d = nc.declare_dram_parameter("whh_t", [HID, 128], bf16, isOutput=False)
    whhb_d = nc.declare_dram_parameter("whh_b", [HID, 128], bf16, isOutput=False)
    h0_d = nc.declare_dram_parameter("h0", [HID, BL], bf16, isOutput=False)
    c0_d = nc.declare_dram_parameter("c0", [HID, BL], f32, isOutput=False)
    fid_d = nc.declare_dram_parameter("fold_id", [128, HID], f32, isOutput=False)
    feat_d = nc.declare_dram_parameter("feat_out", [HID, S], bf16, isOutput=True)
    h_out_d = nc.declare_dram_parameter("h_out", [HID, BL], bf16, isOutput=True)
    c_out_d = nc.declare_dram_parameter("c_out", [HID, BL], f32, isOutput=True)

    with tile.TileContext(nc) as tc:
        with (
            tc.tile_pool(name="const", bufs=1) as cpool,
            tc.tile_pool(name="gx", bufs=1) as gxpool,
            tc.tile_pool(name="state", bufs=1) as spool,
        ):
            # --- weights / constants, loaded once ---
            wca = cpool.tile([KA, FCONV], bf16)
            wcb = cpool.tile([KB, FCONV], bf16)
            wih0 = cpool.tile([128, 256], bf16)
            wih1 = cpool.tile([128, 256], bf16)
            wih2 = cpool.tile([39, 256], bf16)
            whht = cpool.tile([HID, 128], bf16)
            whhb = cpool.tile([HID, 128], bf16)
            h0 = cpool.tile([HID, BL], bf16)
            fid = cpool.tile([128, HID], f32)
            nc.sync.dma_start(out=fid[:], in_=fid_d[:])
            nc.sync.dma_start(out=wca[:], in_=wca_d[:])
            nc.sync.dma_start(out=wcb[:], in_=wcb_d[:])
            nc.sync.dma_start(out=wih0[:], in_=wih0_d[:])
            nc.sync.dma_start(out=wih1[:], in_=wih1_d[:])
            nc.sync.dma_start(out=wih2[:], in_=wih2_d[:])
            nc.sync.dma_start(out=whht[:], in_=whht_d[:])
            nc.sync.dma_start(out=whhb[:], in_=whhb_d[:])
            nc.sync.dma_start(out=h0[:], in_=h0_d[:])

            # persistent gate pre-activations for the whole sequence
            gx_top = gxpool.tile([128, S], f32)   # gates i,f
            gx_bot = gxpool.tile([128, S], f32)   # gates g,o

            # c (rows 0:64, persistent) stacked over g~ (rows 64:128, per step)
            gc = spool.tile([128, BL], f32)
            nc.sync.dma_start(out=gc[0:HID, :], in_=c0_d[:])

            # ---------------- Phase A: conv + einsum -> gx ----------------
            with (
                tc.tile_pool(name="xa", bufs=3) as xapool,
                tc.tile_pool(name="xb", bufs=3) as xbpool,
                tc.tile_pool(name="f01", bufs=2) as fpool,
                tc.tile_pool(name="f2", bufs=2) as f2pool,
                tc.tile_pool(name="pc", bufs=2, space="PSUM") as pcpool,
                tc.tile_pool(name="pg", bufs=2, space="PSUM") as pgpool,
            ):
                for it in range(NTILES):
                    n0 = it * NT
                    xa = xapool.tile([KA, NT], bf16)
                    xb = xbpool.tile([KB, NT], bf16)
                    nc.sync.dma_start(out=xa[:], in_=xT_d[0:KA, n0:n0 + NT])
                    nc.sync.dma_start(out=xb[:], in_=xT_d[KA:KA + KB, n0:n0 + NT])

                    f0 = fpool.tile([128, NT], bf16, tag="f0")
                    f1 = fpool.tile([128, NT], bf16, tag="f1")
                    f2 = f2pool.tile([39, NT], bf16)
                    nc.sync.dma_start(out=f2[32:39, :], in_=ex_d[:, n0:n0 + NT])

                    # conv output M chunks: 0:128 -> f0, 128:256 -> f1, 256:288 -> f2[0:32]
                    for (m0, mc, dst) in ((0, 128, f0[:, :]), (128, 128, f1[:, :]),
                                          (256, 32, f2[0:32, :])):
                        pc = pcpool.tile([mc, NT], f32, tag="pc")
                        nc.tensor.matmul(pc[:], wca[:, m0:m0 + mc], xa[:],
                                         start=True, stop=False)
                        nc.tensor.matmul(pc[:], wcb[:, m0:m0 + mc], xb[:],
                                         start=False, stop=True)
                        nc.scalar.activation(dst, pc[:], AF.Relu)

                    for gh, gdst in ((0, gx_top), (1, gx_bot)):
                        pg = pgpool.tile([128, NT], f32, tag="pg")
                        c0_ = gh * 128
                        nc.tensor.matmul(pg[:], wih0[:, c0_:c0_ + 128], f0[:],
                                         start=True, stop=False)
                        nc.tensor.matmul(pg[:], wih1[:, c0_:c0_ + 128], f1[:],
                                         start=False, stop=False)
                        nc.tensor.matmul(pg[:], wih2[:, c0_:c0_ + 128], f2[:],
                                         start=False, stop=True)
                        nc.vector.tensor_copy(gdst[:, n0:n0 + NT], pg[:])

            # ---------------- Phase B: serial LSTM ----------------
            with (
                tc.tile_pool(name="hs", bufs=2) as hspool,
                tc.tile_pool(name="sb", bufs=3) as sbpool,
                tc.tile_pool(name="pt", bufs=3, space="PSUM") as ptpool,
                tc.tile_pool(name="pb", bufs=3, space="PSUM") as pbpool,
                tc.tile_pool(name="pc", bufs=2, space="PSUM") as pcnpool,
            ):
                h_prev = h0
                hs = None
                for t in range(T):
                    sl = t % CHUNK
                    if sl == 0:
                        hs = hspool.tile([HID, CHUNK * BL], bf16, tag="hs")
                    cl = t * BL

                    # gates: top = [f | i], bot = [o | g]  (host-permuted)
                    pt = ptpool.tile([128, BL], f32, tag="pt")
                    pb = pbpool.tile([128, BL], f32, tag="pb")
                    if USE_PSUM_PRELOAD:
                        nc.vector.tensor_copy(pb[:], gx_bot[:, cl:cl + BL])
                        nc.vector.tensor_copy(pt[:], gx_top[:, cl:cl + BL])
                        # bot first: tanh(g) can start while the top MM runs
                        nc.tensor.matmul(pb[:], whhb[:], h_prev,
                                         start=False, stop=True,
                                         skip_group_check=True)
                        nc.tensor.matmul(pt[:], whht[:], h_prev,
                                         start=False, stop=True,
                                         skip_group_check=True)
                        g_t, g_b = pt, pb
                    else:
                        nc.tensor.matmul(pt[:], whht[:], h_prev,
                                         start=True, stop=True)
                        nc.tensor.matmul(pb[:], whhb[:], h_prev,
                                         start=True, stop=True)
                        at = sbpool.tile([128, BL], f32, tag="at")
                        ab = sbpool.tile([128, BL], f32, tag="ab")
                        nc.vector.tensor_add(at[:], pt[:], gx_top[:, cl:cl + BL])
                        nc.vector.tensor_add(ab[:], pb[:], gx_bot[:, cl:cl + BL])
                        g_t, g_b = at, ab

                    # g~ = tanh(g) into gc rows 64:128 (c sits in rows 0:64)
                    nc.scalar.activation(gc[HID:, :], g_b[HID:, :], AF.Tanh)
                    sig_fi = sbpool.tile([128, BL], f32, tag="sfi")
                    nc.scalar.activation(sig_fi[:], g_t[:], AF.Sigmoid)
                    sig_o = sbpool.tile([HID, BL], f32, tag="so")
                    nc.scalar.activation(sig_o[:], g_b[0:HID, :], AF.Sigmoid)

                    # prod = [f*c | i*g~]; fold halves with [I;I] matmul -> c_new
                    prod = sbpool.tile([128, BL], f32, tag="prod")
                    nc.vector.tensor_mul(prod[:], sig_fi[:], gc[:])
                    pcn = pcnpool.tile([HID, BL], f32, tag="pcn")
                    nc.tensor.matmul(pcn[:], fid[:], prod[:], start=True, stop=True)

                    tc_ = sbpool.tile([HID, BL], f32, tag="tc")
                    nc.scalar.activation(tc_[:], pcn[:], AF.Tanh)
                    hsl = hs[:, sl * BL:(sl + 1) * BL]
                    nc.vector.tensor_mul(hsl, sig_o[:], tc_[:])
                    nc.vector.tensor_copy(gc[0:HID, :], pcn[:])
                    h_prev = hsl

                    if sl == CHUNK - 1:
                        c0o = (t - sl) * BL
                        nc.sync.dma_start(out=feat_d[:, c0o:c0o + CHUNK * BL],
                                          in_=hs[:])

                nc.sync.dma_start(out=h_out_d[:], in_=h_prev)
                nc.sync.dma_start(out=c_out_d[:], in_=gc[0:HID, :])

    nc.compile()
    return nc


def _pack_weights(conv_w, conv_b, w_ih, w_hh, b_ih, b_hh):
    f32 = np.float32
    wc = np.zeros((FIN + 1, FCONV), dtype=f32)
    cw = np.asarray(conv_w, dtype=f32)
    for co in range(8):
        for ci in range(4):
            for di in range(2):
                for dj in range(2):
                    v = cw[co, ci, di, dj]
                    for io in range(6):
                        for jo in range(6):
                            f_in = (io + di) * 28 + (jo + dj) * 4 + ci
                            f_out = co * 36 + io * 6 + jo
                            wc[f_in, f_out] += v
    wc[FIN, :] = np.repeat(np.asarray(conv_b, dtype=f32), 36)

    # gate-row permutation: torch order [i,f,g,o] -> device order [f,i | o,g]
    perm = np.r_[64:128, 0:64, 192:256, 128:192]
    wih_p = np.asarray(w_ih, dtype=f32)[perm]                         # [256, 294]
    whh_p = np.asarray(w_hh, dtype=f32)[perm]                         # [256, 64]
    bias = (np.asarray(b_ih, dtype=f32) + np.asarray(b_hh, dtype=f32))[perm]
    wihT = np.ascontiguousarray(wih_p.T)                              # [294, 256]
    wih2 = np.concatenate([wihT[256:294], bias[None, :]], axis=0)     # [39, 256]
    whhT = np.ascontiguousarray(whh_p.T)                              # [64, 256]
    fold_id = np.concatenate([np.eye(HID, dtype=f32),
                              np.eye(HID, dtype=f32)], axis=0)        # [128, 64]
    return {
        "wc_a": np.ascontiguousarray(wc[0:KA]).astype(BF16),
        "wc_b": np.ascontiguousarray(wc[KA:]).astype(BF16),
        "wih_0": np.ascontiguousarray(wihT[0:128]).astype(BF16),
        "wih_1": np.ascontiguousarray(wihT[128:256]).astype(BF16),
        "wih_2": np.ascontiguousarray(wih2).astype(BF16),
        "whh_t": np.ascontiguousarray(whhT[:, 0:128]).astype(BF16),
        "whh_b": np.ascontiguousarray(whhT[:, 128:256]).astype(BF16),
        "fold_id": fold_id,
    }


def kernel(x, hidden, prev_action, prev_reward, prev_done,
           conv_w, conv_b, w_ih, w_hh, b_ih, b_hh):
    from concourse import bass_utils

    x = np.asarray(x, dtype=np.float32)
    hidden = np.asarray(hidden, dtype=np.float32)
    prev_action = np.asarray(prev_action)
    prev_reward = np.asarray(prev_reward, dtype=np.float32)
    prev_done = np.asarray(prev_done, dtype=np.float32)

    wpack = _pack_weights(conv_w, conv_b, w_ih, w_hh, b_ih, b_hh)

    in_maps = []
    for m in range(NCORES):
        b0 = m * BL
        xm = x[b0:b0 + BL].reshape(BL, T, FIN)
        xTm = np.empty((FIN + 1, S), dtype=np.float32)
        xTm[0:FIN] = xm.transpose(2, 1, 0).reshape(FIN, S)
        xTm[FIN] = 1.0

        pa = prev_action[b0:b0 + BL].T.reshape(-1)          # [S] t-major
        ex = np.empty((7, S), dtype=np.float32)
        ex[0:4] = (np.arange(NUM_ACT)[:, None] == pa[None, :])
        ex[4] = prev_reward[b0:b0 + BL].T.reshape(-1)
        ex[5] = prev_done[b0:b0 + BL].T.reshape(-1)
        ex[6] = 1.0

        hm = hidden[b0:b0 + BL]
        im = {
            "xT": xTm.astype(BF16),
            "extras": ex.astype(BF16),
            "h0": np.ascontiguousarray(hm[:, 0:HID].T).astype(BF16),
            "c0": np.ascontiguousarray(hm[:, HID:].T),
        }
        im.update(wpack)
        in_maps.append(im)

    if "nc" not in _cache:
        _cache["nc"] = _build()
    nc = _cache["nc"]

    res = bass_utils.run_bass_kernel_spmd(nc, in_maps, core_ids=list(range(NCORES)))

    features = np.empty((B, T, HID), dtype=np.float32)
    hidden_out = np.empty((B, 2 * HID), dtype=np.float32)
    for m in range(NCORES):
        b0 = m * BL
        fo = np.asarray(res.results[m]["feat_out"], dtype=np.float32)
        features[b0:b0 + BL] = fo.reshape(HID, T, BL).transpose(2, 1, 0)
        hidden_out[b0:b0 + BL, 0:HID] = np.asarray(
            res.results[m]["h_out"], dtype=np.float32).T
        hidden_out[b0:b0 + BL, HID:] = res.results[m]["c_out"].T
    return features, hidden_out


# revision 17
# speedup vs baseline: 1.9536x; 1.3280x over previous
"""Trainium2 Bass kernel: conv2x2 + LSTM actor-critic feature trunk.

Full inputs in, full outputs out. Data-parallel over batch: 8 NeuronCores,
32 sequences per core. Per core:
  Phase A: dense-matmul conv (bias folded via ones-row) -> relu -> einsum
           producing gate pre-activations gx[4H, T*32] kept in SBUF.
  Phase B: 512 serial LSTM steps, hidden state stored [H=64 part, B=32 free]
           so the recurrent matmul needs no per-step transpose; gx is
           preloaded into PSUM and the matmul accumulates onto it. The
           cross-partition c-update (f*c + i*g~) is folded with an [I;I]
           matmul on the PE (vector engine cannot cross partition lanes).
Matmul operands (x, conv/ih/hh weights, h) are bf16; PSUM accumulation,
gates, and cell state stay f32. Measured: 1.53 ms, rel_err 2.6e-3.
"""

import sys
import numpy as np
import ml_dtypes

BF16 = ml_dtypes.bfloat16

for _p in ("/opt/trn_rl_repo",):
    if _p not in sys.path:
        sys.path.insert(0, _p)

HID = 64
NUM_ACT = 4
B, T = 256, 512
NCORES = 8
BL = B // NCORES          # 32 sequences per core
S = BL * T                # 16384 samples per core, t-major: s = t*BL + b
NT = 512                  # samples per phase-A tile
NTILES = S // NT
FIN = 7 * 7 * 4           # 196 input features per sample
FCONV = 8 * 6 * 6         # 288 conv output features
FEAT = FCONV + NUM_ACT + 2  # 294
KA = 128                  # phase-A K chunk split of FIN+1: 128 + 69
KB = FIN + 1 - KA
CHUNK = 64                # LSTM steps per output DMA chunk

USE_PSUM_PRELOAD = True

_cache = {}


def _build():
    from concourse import bacc, tile, mybir

    f32 = mybir.dt.float32
    bf16 = mybir.dt.bfloat16
    AF = mybir.ActivationFunctionType

    nc = bacc.Bacc(None, target_bir_lowering=False, debug=False)

    xT_d = nc.declare_dram_parameter("xT", [FIN + 1, S], bf16, isOutput=False)
    ex_d = nc.declare_dram_parameter("extras", [7, S], bf16, isOutput=False)
    wca_d = nc.declare_dram_parameter("wc_a", [KA, FCONV], bf16, isOutput=False)
    wcb_d = nc.declare_dram_parameter("wc_b", [KB, FCONV], bf16, isOutput=False)
    wih0_d = nc.declare_dram_parameter("wih_0", [128, 256], bf16, isOutput=False)
    wih1_d = nc.declare_dram_parameter("wih_1", [128, 256], bf16, isOutput=False)
    wih2_d = nc.declare_dram_parameter("wih_2", [39, 256], bf16, isOutput=False)
    whht_d = nc.declare_dram_parameter("whh_t", [HID, 128], bf16, isOutput=False)
    whhb_d = nc.declare_dram_parameter("whh_b", [HID, 128], bf16, isOutput=False)
    h0_d = nc.declare_dram_parameter("h0", [HID, BL], bf16, isOutput=False)
    c0_d = nc.declare_dram_parameter("c0", [HID, BL], f32, isOutput=False)
    fid_d = nc.declare_dram_parameter("fold_id", [128, HID], bf16, isOutput=False)
    feat_d = nc.declare_dram_parameter("feat_out", [HID, S], bf16, isOutput=True)
    h_out_d = nc.declare_dram_parameter("h_out", [HID, BL], bf16, isOutput=True)
    c_out_d = nc.declare_dram_parameter("c_out", [HID, BL], f32, isOutput=True)

    with tile.TileContext(nc) as tc:
        with (
            tc.tile_pool(name="const", bufs=1) as cpool,
            tc.tile_pool(name="gx", bufs=1) as gxpool,
            tc.tile_pool(name="state", bufs=1) as spool,
        ):
            # --- weights / constants, loaded once ---
            wca = cpool.tile([KA, FCONV], bf16)
            wcb = cpool.tile([KB, FCONV], bf16)
            wih0 = cpool.tile([128, 256], bf16)
            wih1 = cpool.tile([128, 256], bf16)
            wih2 = cpool.tile([39, 256], bf16)
            whht = cpool.tile([HID, 128], bf16)
            whhb = cpool.tile([HID, 128], bf16)
            h0 = cpool.tile([HID, BL], bf16)
            fid = cpool.tile([128, HID], bf16)
            nc.sync.dma_start(out=fid[:], in_=fid_d[:])
            nc.sync.dma_start(out=wca[:], in_=wca_d[:])
            nc.sync.dma_start(out=wcb[:], in_=wcb_d[:])
            nc.sync.dma_start(out=wih0[:], in_=wih0_d[:])
            nc.sync.dma_start(out=wih1[:], in_=wih1_d[:])
            nc.sync.dma_start(out=wih2[:], in_=wih2_d[:])
            nc.sync.dma_start(out=whht[:], in_=whht_d[:])
            nc.sync.dma_start(out=whhb[:], in_=whhb_d[:])
            nc.sync.dma_start(out=h0[:], in_=h0_d[:])

            # persistent gate pre-activations for the whole sequence
            gx_top = gxpool.tile([128, S], f32)   # gates i,f
            gx_bot = gxpool.tile([128, S], f32)   # gates g,o

            # c (rows 0:64, persistent) stacked over g~ (rows 64:128, per step)
            gc = spool.tile([128, BL], f32)
            nc.sync.dma_start(out=gc[0:HID, :], in_=c0_d[:])

            # ---------------- Phase A: conv + einsum -> gx ----------------
            with (
                tc.tile_pool(name="xa", bufs=3) as xapool,
                tc.tile_pool(name="xb", bufs=3) as xbpool,
                tc.tile_pool(name="f01", bufs=2) as fpool,
                tc.tile_pool(name="f2", bufs=2) as f2pool,
                tc.tile_pool(name="pc", bufs=2, space="PSUM") as pcpool,
                tc.tile_pool(name="pg", bufs=2, space="PSUM") as pgpool,
            ):
                for it in range(NTILES):
                    n0 = it * NT
                    xa = xapool.tile([KA, NT], bf16)
                    xb = xbpool.tile([KB, NT], bf16)
                    nc.sync.dma_start(out=xa[:], in_=xT_d[0:KA, n0:n0 + NT])
                    nc.sync.dma_start(out=xb[:], in_=xT_d[KA:KA + KB, n0:n0 + NT])

                    f0 = fpool.tile([128, NT], bf16, tag="f0")
                    f1 = fpool.tile([128, NT], bf16, tag="f1")
                    f2 = f2pool.tile([39, NT], bf16)
                    nc.sync.dma_start(out=f2[32:39, :], in_=ex_d[:, n0:n0 + NT])

                    # conv output M chunks: 0:128 -> f0, 128:256 -> f1, 256:288 -> f2[0:32]
                    for (m0, mc, dst) in ((0, 128, f0[:, :]), (128, 128, f1[:, :]),
                                          (256, 32, f2[0:32, :])):
                        pc = pcpool.tile([mc, NT], f32, tag="pc")
                        nc.tensor.matmul(pc[:], wca[:, m0:m0 + mc], xa[:],
                                         start=True, stop=False)
                        nc.tensor.matmul(pc[:], wcb[:, m0:m0 + mc], xb[:],
                                         start=False, stop=True)
                        nc.scalar.activation(dst, pc[:], AF.Relu)

                    for gh, gdst in ((0, gx_top), (1, gx_bot)):
                        pg = pgpool.tile([128, NT], f32, tag="pg")
                        c0_ = gh * 128
                        nc.tensor.matmul(pg[:], wih0[:, c0_:c0_ + 128], f0[:],
                                         start=True, stop=False)
                        nc.tensor.matmul(pg[:], wih1[:, c0_:c0_ + 128], f1[:],
                                         start=False, stop=False)
                        nc.tensor.matmul(pg[:], wih2[:, c0_:c0_ + 128], f2[:],
                                         start=False, stop=True)
                        nc.vector.tensor_copy(gdst[:, n0:n0 + NT], pg[:])

            # ---------------- Phase B: serial LSTM ----------------
            with (
                tc.tile_pool(name="hs", bufs=2) as hspool,
                tc.tile_pool(name="sb", bufs=3) as sbpool,
                tc.tile_pool(name="pt", bufs=3, space="PSUM") as ptpool,
                tc.tile_pool(name="pb", bufs=3, space="PSUM") as pbpool,
                tc.tile_pool(name="pc", bufs=2, space="PSUM") as pcnpool,
            ):
                h_prev = h0
                hs = None
                for t in range(T):
                    sl = t % CHUNK
                    if sl == 0:
                        hs = hspool.tile([HID, CHUNK * BL], bf16, tag="hs")
                    cl = t * BL

                    # gates: top = [f | i], bot = [o | g]  (host-permuted)
                    pt = ptpool.tile([128, BL], f32, tag="pt")
                    pb = pbpool.tile([128, BL], f32, tag="pb")
                    if USE_PSUM_PRELOAD:
                        nc.vector.tensor_copy(pb[:], gx_bot[:, cl:cl + BL])
                        nc.vector.tensor_copy(pt[:], gx_top[:, cl:cl + BL])
                        # bot first: tanh(g) can start while the top MM runs
                        nc.tensor.matmul(pb[:], whhb[:], h_prev,
                                         start=False, stop=True,
                                         skip_group_check=True)
                        nc.tensor.matmul(pt[:], whht[:], h_prev,
                                         start=False, stop=True,
                                         skip_group_check=True)
                        g_t, g_b = pt, pb
                    else:
                        nc.tensor.matmul(pt[:], whht[:], h_prev,
                                         start=True, stop=True)
                        nc.tensor.matmul(pb[:], whhb[:], h_prev,
                                         start=True, stop=True)
                        at = sbpool.tile([128, BL], f32, tag="at")
                        ab = sbpool.tile([128, BL], f32, tag="ab")
                        nc.vector.tensor_add(at[:], pt[:], gx_top[:, cl:cl + BL])
                        nc.vector.tensor_add(ab[:], pb[:], gx_bot[:, cl:cl + BL])
                        g_t, g_b = at, ab

                    # g~ = tanh(g) into gc rows 64:128 (c sits in rows 0:64)
                    nc.scalar.activation(gc[HID:, :], g_b[HID:, :], AF.Tanh)
                    sig_fi = sbpool.tile([128, BL], f32, tag="sfi")
                    nc.scalar.activation(sig_fi[:], g_t[:], AF.Sigmoid)
                    sig_o = sbpool.tile([HID, BL], f32, tag="so")
                    nc.scalar.activation(sig_o[:], g_b[0:HID, :], AF.Sigmoid)

                    # prod = [f*c | i*g~]; fold halves with [I;I] matmul -> c_new
                    prod = sbpool.tile([128, BL], bf16, tag="prod")
                    nc.vector.tensor_mul(prod[:], sig_fi[:], gc[:])
                    pcn = pcnpool.tile([HID, BL], f32, tag="pcn")
                    nc.tensor.matmul(pcn[:], fid[:], prod[:], start=True, stop=True)

                    tc_ = sbpool.tile([HID, BL], f32, tag="tc")
                    nc.scalar.activation(tc_[:], pcn[:], AF.Tanh)
                    hsl = hs[:, sl * BL:(sl + 1) * BL]
                    nc.vector.tensor_mul(hsl, sig_o[:], tc_[:])
                    nc.vector.tensor_copy(gc[0:HID, :], pcn[:])
                    h_prev = hsl

                    if sl == CHUNK - 1:
                        c0o = (t - sl) * BL
                        nc.sync.dma_start(out=feat_d[:, c0o:c0o + CHUNK * BL],
                                          in_=hs[:])

                nc.sync.dma_start(out=h_out_d[:], in_=h_prev)
                nc.sync.dma_start(out=c_out_d[:], in_=gc[0:HID, :])

    nc.compile()
    return nc


def _pack_weights(conv_w, conv_b, w_ih, w_hh, b_ih, b_hh):
    f32 = np.float32
    wc = np.zeros((FIN + 1, FCONV), dtype=f32)
    cw = np.asarray(conv_w, dtype=f32)
    for co in range(8):
        for ci in range(4):
            for di in range(2):
                for dj in range(2):
                    v = cw[co, ci, di, dj]
                    for io in range(6):
                        for jo in range(6):
                            f_in = (io + di) * 28 + (jo + dj) * 4 + ci
                            f_out = co * 36 + io * 6 + jo
                            wc[f_in, f_out] += v
    wc[FIN, :] = np.repeat(np.asarray(conv_b, dtype=f32), 36)

    # gate-row permutation: torch order [i,f,g,o] -> device order [f,i | o,g]
    perm = np.r_[64:128, 0:64, 192:256, 128:192]
    wih_p = np.asarray(w_ih, dtype=f32)[perm]                         # [256, 294]
    whh_p = np.asarray(w_hh, dtype=f32)[perm]                         # [256, 64]
    bias = (np.asarray(b_ih, dtype=f32) + np.asarray(b_hh, dtype=f32))[perm]
    wihT = np.ascontiguousarray(wih_p.T)                              # [294, 256]
    wih2 = np.concatenate([wihT[256:294], bias[None, :]], axis=0)     # [39, 256]
    whhT = np.ascontiguousarray(whh_p.T)                              # [64, 256]
    fold_id = np.concatenate([np.eye(HID, dtype=f32),
                              np.eye(HID, dtype=f32)], axis=0)        # [128, 64]
    return {
        "wc_a": np.ascontiguousarray(wc[0:KA]).astype(BF16),
        "wc_b": np.ascontiguousarray(wc[KA:]).astype(BF16),
        "wih_0": np.ascontiguousarray(wihT[0:128]).astype(BF16),
        "wih_1": np.ascontiguousarray(wihT[128:256]).astype(BF16),
        "wih_2": np.ascontiguousarray(wih2).astype(BF16),
        "whh_t": np.ascontiguousarray(whhT[:, 0:128]).astype(BF16),
        "whh_b": np.ascontiguousarray(whhT[:, 128:256]).astype(BF16),
        "fold_id": fold_id.astype(BF16),
    }


def kernel(x, hidden, prev_action, prev_reward, prev_done,
           conv_w, conv_b, w_ih, w_hh, b_ih, b_hh):
    from concourse import bass_utils

    x = np.asarray(x, dtype=np.float32)
    hidden = np.asarray(hidden, dtype=np.float32)
    prev_action = np.asarray(prev_action)
    prev_reward = np.asarray(prev_reward, dtype=np.float32)
    prev_done = np.asarray(prev_done, dtype=np.float32)

    wpack = _pack_weights(conv_w, conv_b, w_ih, w_hh, b_ih, b_hh)

    in_maps = []
    for m in range(NCORES):
        b0 = m * BL
        xm = x[b0:b0 + BL].reshape(BL, T, FIN)
        xTm = np.empty((FIN + 1, S), dtype=np.float32)
        xTm[0:FIN] = xm.transpose(2, 1, 0).reshape(FIN, S)
        xTm[FIN] = 1.0

        pa = prev_action[b0:b0 + BL].T.reshape(-1)          # [S] t-major
        ex = np.empty((7, S), dtype=np.float32)
        ex[0:4] = (np.arange(NUM_ACT)[:, None] == pa[None, :])
        ex[4] = prev_reward[b0:b0 + BL].T.reshape(-1)
        ex[5] = prev_done[b0:b0 + BL].T.reshape(-1)
        ex[6] = 1.0

        hm = hidden[b0:b0 + BL]
        im = {
            "xT": xTm.astype(BF16),
            "extras": ex.astype(BF16),
            "h0": np.ascontiguousarray(hm[:, 0:HID].T).astype(BF16),
            "c0": np.ascontiguousarray(hm[:, HID:].T),
        }
        im.update(wpack)
        in_maps.append(im)

    if "nc" not in _cache:
        _cache["nc"] = _build()
    nc = _cache["nc"]

    res = bass_utils.run_bass_kernel_spmd(nc, in_maps, core_ids=list(range(NCORES)))

    features = np.empty((B, T, HID), dtype=np.float32)
    hidden_out = np.empty((B, 2 * HID), dtype=np.float32)
    for m in range(NCORES):
        b0 = m * BL
        fo = np.asarray(res.results[m]["feat_out"], dtype=np.float32)
        features[b0:b0 + BL] = fo.reshape(HID, T, BL).transpose(2, 1, 0)
        hidden_out[b0:b0 + BL, 0:HID] = np.asarray(
            res.results[m]["h_out"], dtype=np.float32).T
        hidden_out[b0:b0 + BL, HID:] = res.results[m]["c_out"].T
    return features, hidden_out
